# revision 53
# baseline (speedup 1.0000x reference)
"""Trainium2 Bass kernel for batched displacement-operator construction.

Math: Da[b] = diag(u) (V diag(exp(-i r lam)) V^T) diag(conj(u)) with
u_i = w^i, w = i*alpha/|alpha|.  Three structural reductions vs the
dense reference:

1. Parity: the generator is bipartite, so lam_{N-1-k} = -lam_k and
   v_{N-1-k}(i) = +-(-1)^i v_k(i).  Folding each +-lam pair, E_ij is
   REAL = sum_{k<512} v_ik v_jk 2cos(r lam_k) on i+j even and pure
   IMAGINARY = -i sum v_ik v_jk 2sin(r lam_k) on i+j odd: two
   half-contraction (512) fp16 matmuls, and the w^(i-j) phase multiply
   needs just 2 real mults per output element.
2. Band: |Da_ij| is negligible beyond |i-j| > 2 r sqrt(N) ~ 330
   (verified 2e-6 of the Frobenius norm at W=384), so only column
   windows around the diagonal are computed.
3. Symmetry: C and S are symmetric, so Da(i>j) is a parity-signed copy
   of Da(j<i).  The device computes the upper-triangular band only;
   the host mirrors the lower half while converting fp16 -> complex64.

Device layout: rows parity-grouped (pi), columns parity-packed (pj).
Per row-chunk mc the packed column window is [128*mc, 128*mc + w),
w = [320, 320, 256, 128], which makes the Toeplitz phase-table slice
offset chunk-independent (tables are [128, 320] per kind).  Moving
operands are per-alpha scaled packed V^T halves (DVE-4x / ACT builds);
matmuls run fp16 at 1 cycle/row with fp32 PSUM; the phase multiply is
one tensor_tensor per (tile, Re/Im) from PSUM into parity-interleaved
fp16 tiles (split DVE/Pool by a greedy load balance); output DMAs are
contiguous >=512B descriptors into parity-grouped DRAM.

Sharding: 16 alphas data-parallel over 8 cores (2 per core).
"""

import sys

sys.path.insert(0, "/opt/trn_rl_repo")

import numpy as np

N = 1024
B = 16
NCORES = 8
APC = B // NCORES  # alphas per core
P = 128
H = N // 2  # half eigenbasis / parity-packed size
KC = H // P  # contraction chunks (4)
MC = H // P  # row chunks per parity (4)
WW = 336  # phase-table stride (>= 321 used columns, padded)
WIDTHS = (320, 320, 256, 128)  # packed column window per row chunk
NTAB = 4  # phase tables per alpha: RE, IE, RO(+1), IO(+1)

_cache = {}


def _build_module(reps=1):
    import contextlib

    import concourse.bacc as bacc
    import concourse.mybir as mybir
    import concourse.tile as tile

    f16 = mybir.dt.float16
    f32 = mybir.dt.float32

    nc = bacc.Bacc(
        "TRN2",
        target_bir_lowering=False,
        debug=False,
        num_devices=NCORES,
    )

    # esc (cc/ss per-partition fp32 scalars) rides in the first vth
    # chunk's DMA as raw bytes: 2*ESCW fp16 slots bitcast back to fp32.
    ESCW = APC * 2 * KC
    vth_d = nc.dram_tensor("vth", [P, 2 * KC * H + 2 * ESCW], f16,
                           kind="ExternalInput")
    ph_d = nc.dram_tensor("ph", [P, APC * NTAB * WW], f16, kind="ExternalInput")
    # packed output: [alpha, row-parity pi, row, Re/Im, col-parity pj, c]
    out_d = nc.dram_tensor("out", [APC, 2, H, 2, 2, WW], f16,
                           kind="ExternalOutput")

    with tile.TileContext(nc) as tc:
        with (
            tc.tile_pool(name="const", bufs=1) as cpool,
            tc.tile_pool(name="wts", bufs=2) as wpool,
            tc.tile_pool(name="evac", bufs=6) as epool,
            tc.tile_pool(name="outp", bufs=3) as outp,
            tc.tile_pool(name="psum", bufs=2, space="PSUM") as pp,
        ):
            ph = cpool.tile([P, APC * NTAB * WW], f16)
            # Four separate chunk-pair tiles so readers dep-track at DMA
            # granularity (slices of one big tile wait on all writers).
            # vq[0] carries the esc scalars in its tail columns.
            vq = [cpool.tile([P, 2 * H + (2 * ESCW if i == 0 else 0)], f16,
                             tag=f"vq{i}", name=f"vq{i}")
                  for i in range(4)]  # [e01+esc, o01, e23, o23]
            esc = vq[0][:, 2 * H :].bitcast(f32)
            vthe = [vq[0][:, :H], vq[0][:, H : 2 * H],
                    vq[2][:, :H], vq[2][:, H:]]
            vtho = [vq[1][:, :H], vq[1][:, H:], vq[3][:, :H], vq[3][:, H:]]

            # Warm the ACT Copy-function table at t=0 so the implicit
            # ACT_TABLE_LOAD doesn't push the first PSUM evacuation out
            # in the scheduler's model.
            dummy = cpool.tile([P, 1], mybir.dt.float32, name="dummy")
            nc.gpsimd.memset(dummy[:], 0.0)
            nc.scalar.activation(dummy[:], dummy[:],
                                 mybir.ActivationFunctionType.Copy)

            # One HWDGE chain ordered by first use (DMA_ENGINES
            # serializes transfers, so order = availability order).
            # vth_d column layout: [e01 | e23 | o01 | o23] (pe-major).
            # vth_d layout: [e01 | esc | o01 | e23 | o23].  Transfer
            # order = first-use order: all four v-chunks gate the mc0
            # accumulation loop; alpha-0 phase tables next; alpha-1 last.
            E = 2 * H + 2 * ESCW
            nc.sync.dma_start(vq[0][:], vth_d[:, 0:E])
            nc.sync.dma_start(ph[:, : 2 * WW], ph_d[:, : 2 * WW])
            nc.sync.dma_start(vq[1][:], vth_d[:, E : E + 2 * H])
            nc.sync.dma_start(vq[2][:], vth_d[:, E + 2 * H : E + 4 * H])
            nc.sync.dma_start(vq[3][:], vth_d[:, E + 4 * H : E + 6 * H])
            nc.sync.dma_start(ph[:, 2 * WW : 4 * WW], ph_d[:, 2 * WW : 4 * WW])
            if APC > 1:
                s = NTAB * WW
                nc.sync.dma_start(ph[:, s : s + NTAB * WW],
                                  ph_d[:, s : s + NTAB * WW])

            if _cache.get("unroll"):
                for _ in range(reps):
                    _emit_body(nc, tc, vthe, vtho, esc, ph, wpool, epool,
                               outp, pp, out_d, mybir)
            else:
                rep_ctx = (
                    tc.For_i(0, reps, 1) if reps > 1
                    else contextlib.nullcontext()
                )
                with rep_ctx:
                    _emit_body(nc, tc, vthe, vtho, esc, ph, wpool, epool,
                               outp, pp, out_d, mybir)

    nc.compile()
    return nc


def _emit_body(nc, tc, vthe, vtho, esc, ph, wpool, epool, outp, pp,
               out_d, mybir):
    f16 = mybir.dt.float16
    f32 = mybir.dt.float32
    Alu = mybir.AluOpType
    Act = mybir.ActivationFunctionType

    T_RE, T_IE, T_RO, T_IO = range(NTAB)

    # Greedy DVE/Pool balance for the phase multiplies.  All-SBUF fp16
    # packed operands: DVE gets the 2x_1p mode (~0.52 ns/row), Pool
    # (GPSIMD) runs at ~1.98 ns/row but is otherwise idle.
    load = {"v": 0.0, "g": 0.0}

    def tt(dst, src0, src1):
        cv = src0.shape[-1] * 0.521 + 120 + load["v"]
        cg = src0.shape[-1] * 1.984 + 140 + load["g"]
        if cv <= cg:
            load["v"] = cv
            nc.vector.tensor_tensor(dst, src0, src1, Alu.mult)
        else:
            load["g"] = cg
            nc.gpsimd.tensor_tensor(dst, src0, src1, Alu.mult)

    for a in range(APC):
        cc = [a * 2 * KC + kc for kc in range(KC)]
        ss = [a * 2 * KC + KC + kc for kc in range(KC)]

        # Moving tables: A = cc*vthe, Bt = ss*vtho, Ct = ss*vthe,
        # Dt = cc*vtho — all on DVE (4x fp16 tensor_scalar); ACT is
        # saturated by the PSUM evacuations.
        A = [wpool.tile([P, H], f16, tag=f"A{kc}", name=f"A{kc}_{a}")
             for kc in range(KC)]
        Bt = [wpool.tile([P, H], f16, tag=f"B{kc}", name=f"B{kc}_{a}")
              for kc in range(KC)]
        Ct = [wpool.tile([P, H], f16, tag=f"C{kc}", name=f"C{kc}_{a}")
              for kc in range(KC)]
        Dt = [wpool.tile([P, H], f16, tag=f"D{kc}", name=f"D{kc}_{a}")
              for kc in range(KC)]
        # pi=0 needs only A and Bt; Ct/Dt are emitted after the first
        # output block so they don't delay the first phase multiplies.
        for kc in range(KC):
            nc.vector.tensor_scalar_mul(
                A[kc][:], vthe[kc][:], esc[:, cc[kc] : cc[kc] + 1]
            )
            nc.vector.tensor_scalar_mul(
                Bt[kc][:], vtho[kc][:], esc[:, ss[kc] : ss[kc] + 1]
            )
        load["v"] += 16 * 200 / APC  # builds preload the DVE estimate

        for pi in range(2):
            stat = vthe if pi == 0 else vtho
            movE = A if pi == 0 else Ct  # same-parity cols (C values)
            movO = Bt if pi == 0 else Dt  # cross-parity cols (S values)
            abase = a * NTAB * WW
            # (pair base table, +1-shift) per pj.  (RE, IE) and (RO, IO)
            # are adjacent, so one broadcast tensor_tensor computes the
            # Re AND Im planes of a z tile; odd-d tables are shared with
            # the pi=0 (d = 2q-1) variant a one-column shift.
            sh = 1 if pi == 0 else 0
            tabs = [(T_RE, 0), (T_RO, sh)] if pi == 0 else \
                   [(T_RO, sh), (T_RE, 0)]

            # Interleaved per-mc emission: output tiles complete at an
            # even rate so out-DMAs stream instead of bunching at the
            # end (DMA_ENGINES is near-saturated).
            # Last pass runs mc descending so the final tile (and its
            # DMA) is the smallest -> shortest pipeline drain.
            mcs = (range(MC) if not (a == APC - 1 and pi == 1)
                   else range(MC - 1, -1, -1))
            for mc in mcs:
                w = WIDTHS[mc]
                c0 = P * mc
                to = outp.tile([P, 2, 2, w], f16, tag=f"to{mc}")
                # zE sub-block then zO sub-block: the evac + pj=0 phase
                # multiplies overlap the zO matmuls, so output tiles and
                # their DMAs start ~2 tiles earlier (ACT evacuates PSUM
                # -> SBUF fp16; GPSIMD cannot touch PSUM, and fp16 SBUF
                # operands give DVE the 2x mode).
                for pj, mov, ztag, etag in (
                    (0, movE, "zE", "zsE"), (1, movO, "zO", "zsO"),
                ):
                    z = pp.tile([P, w], f32, tag=ztag)
                    for kc in range(KC):
                        sap = stat[kc][:, mc * P : (mc + 1) * P]
                        nc.tensor.matmul(z[:], sap, mov[kc][:, c0 : c0 + w],
                                         start=kc == 0, stop=kc == KC - 1)
                    zs = epool.tile([P, w], f16, tag=etag)
                    nc.scalar.activation(zs[:], z[:], Act.Copy)
                    tb, s = tabs[pj]
                    bR = abase + tb * WW + s
                    bI = abase + (tb + 1) * WW + s
                    tt(to[:, 0, pj, :], zs[:], ph[:, bR : bR + w])
                    tt(to[:, 1, pj, :], zs[:], ph[:, bI : bI + w])
                nc.sync.dma_start(
                    out_d[a, pi, mc * P : (mc + 1) * P, :, :, 0:w],
                    to[:],
                )
                if pi == 0 and mc == 0:
                    for kc in range(KC):
                        nc.vector.tensor_scalar_mul(
                            Ct[kc][:], vthe[kc][:],
                            esc[:, ss[kc] : ss[kc] + 1],
                        )
                        nc.vector.tensor_scalar_mul(
                            Dt[kc][:], vtho[kc][:],
                            esc[:, cc[kc] : cc[kc] + 1],
                        )


def _get_module():
    if "nc" not in _cache:
        _cache["nc"] = _build_module()
    return _cache["nc"]


def _host_precompute(alpha_real, alpha_imag, evals):
    """Per-alpha scalar/phase tables, mirroring the reference's fp32 path."""
    ar = np.asarray(alpha_real, np.float32)
    ai = np.asarray(alpha_imag, np.float32)
    ev = np.asarray(evals, np.float32)

    esc_all = np.empty((B, 2, KC, P), np.float32)  # (b, cc/ss, kc, p)
    ph_all = np.empty((B, NTAB, P, WW), np.float16)

    prow = np.arange(P)[:, None]
    ucol = np.arange(WW)[None, :]
    q = prow - ucol  # q in [-335, 127]
    dE = 2 * q + (N - 1)  # indices into d-table of length 2N-1
    dOp = 2 * q + 1 + (N - 1)

    for b in range(B):
        alpha = np.complex64(complex(ar[b], ai[b]))
        r = np.float32(np.abs(alpha)) + np.float32(1e-10)
        eit = np.complex64(alpha / r)
        w = np.complex128(1j) * np.complex128(eit)

        t32 = (np.float32(r) * ev[:H]).astype(np.float32)
        t64 = t32.astype(np.float64)
        esc_all[b, 0] = (2.0 * np.cos(t64)).astype(np.float32).reshape(KC, P)
        esc_all[b, 1] = (2.0 * np.sin(t64)).astype(np.float32).reshape(KC, P)

        d = np.arange(-(N - 1), N)
        ptab = w ** d  # complex128, |w|~1
        wc = ptab.real.astype(np.float32)
        ws = ptab.imag.astype(np.float32)
        ph_all[b, 0] = wc[dE]  # T_RE: cos at even d = 2(p-u)
        ph_all[b, 1] = ws[dE]  # T_IE: sin at even d
        ph_all[b, 2] = ws[dOp]  # T_RO: sin at odd d = 2(p-u)+1
        ph_all[b, 3] = -wc[dOp]  # T_IO: -cos at odd d

    return esc_all, ph_all


def _build_in_maps(alpha_real, alpha_imag, evals, evecs):
    evecs_f = np.asarray(evecs, np.float32)
    Vh = evecs_f[:, :H].astype(np.float16)
    ESCW = APC * 2 * KC
    # chunk blocks: blk[pe][kc][p, c] = V[2c + pe, kc*P + p]
    blks = {(pe, kc): Vh[pe::2, kc * P : (kc + 1) * P].T
            for pe in range(2) for kc in range(KC)}
    esc_all, ph_all = _host_precompute(alpha_real, alpha_imag, evals)

    in_maps = []
    for c in range(NCORES):
        bs = [c * APC + a for a in range(APC)]
        esc = np.empty((P, ESCW), np.float32)
        ph = np.empty((P, APC * NTAB * WW), np.float16)
        for a, b in enumerate(bs):
            for which in range(2):
                cols = a * 2 * KC + which * KC
                esc[:, cols : cols + KC] = esc_all[b, which].T
            for t in range(NTAB):
                wbase = (a * NTAB + t) * WW
                ph[:, wbase : wbase + WW] = ph_all[b, t]
        # vth_d layout: [e01 | esc-bytes | o01 | e23 | o23]
        vth = np.empty((P, 2 * KC * H + 2 * ESCW), np.float16)
        vth[:, 0:H] = blks[(0, 0)]
        vth[:, H : 2 * H] = blks[(0, 1)]
        vth[:, 2 * H : 2 * H + 2 * ESCW] = esc.view(np.float16)
        E = 2 * H + 2 * ESCW
        vth[:, E : E + H] = blks[(1, 0)]
        vth[:, E + H : E + 2 * H] = blks[(1, 1)]
        vth[:, E + 2 * H : E + 3 * H] = blks[(0, 2)]
        vth[:, E + 3 * H : E + 4 * H] = blks[(0, 3)]
        vth[:, E + 4 * H : E + 5 * H] = blks[(1, 2)]
        vth[:, E + 5 * H : E + 6 * H] = blks[(1, 3)]
        in_maps.append({"vth": vth, "ph": ph})
    return in_maps


_masks = {}


def _get_masks():
    if not _masks:
        ii = np.arange(N)[:, None]
        jj = np.arange(N)[None, :]
        _masks["upper"] = jj >= ii
        _masks["sre"] = np.where((ii + jj) % 2 == 0, np.float32(1), np.float32(-1))
    return _masks["upper"], _masks["sre"]


def _assemble(od, out, b):
    """Un-pack parities/windows, mirror the lower triangle, complex64.

    od: [2(pi), H, 2(ri), 2(pj), WW] fp16 device output for one alpha.
    """
    upper, sre = _get_masks()
    cre = np.zeros((N, N), np.float32)
    cim = np.zeros((N, N), np.float32)
    for pi in range(2):
        rows = cre[pi::2], cim[pi::2]
        for mc in range(MC):
            w = WIDTHS[mc]
            c0 = P * mc
            blk = od[pi, mc * P : (mc + 1) * P, :, :, :w].astype(np.float32)
            for pj in range(2):
                rows[0][mc * P : (mc + 1) * P,
                        2 * c0 + pj : 2 * (c0 + w) + pj : 2] = blk[:, 0, pj]
                rows[1][mc * P : (mc + 1) * P,
                        2 * c0 + pj : 2 * (c0 + w) + pj : 2] = blk[:, 1, pj]
    out.real[b] = np.where(upper, cre, sre * cre.T)
    out.imag[b] = np.where(upper, cim, -sre * cim.T)


def kernel(alpha_real, alpha_imag, evals, evecs):
    from concourse import bass_utils

    nc = _get_module()
    in_maps = _build_in_maps(alpha_real, alpha_imag, evals, evecs)

    res = bass_utils.run_bass_kernel_spmd(
        nc, in_maps, core_ids=list(range(NCORES))
    )

    out = np.empty((B, N, N), np.complex64)
    for c in range(NCORES):
        od = res.results[c]["out"]  # [APC, 2, H, 2, 2, WW]
        for a in range(APC):
            _assemble(od[a], out, c * APC + a)
    return out
